# revision 22
# baseline (speedup 1.0000x reference)
"""Causal self-attention (GQA + RoPE + QK-RMSNorm) on 8 trn2 NeuronCores.

Sharding: core c -> (batch b = c//2, parity g = c%2).  Each core computes
the outputs for 512 of its batch's 1024 query tokens (4 q-blocks of 128
tokens; parity 0 takes absolute blocks {0,3,4,7}, parity 1 {1,2,5,6}) over
all 16 heads, plus the full K/V for the batch.  No collectives; the host
gathers/scatters rows only.

Engine plan (v2):
- Projections emit Q^T/K^T with head_dim on partitions.  RMSNorm scale is
  one Dsqrt activation (rsqrt via 0.5/sqrt(x/4)) + a broadcast matmul;
  RoPE uses the swap formulation (x*[c;c] + swap(x)*[-s;s]) = 5 DVE ops.
- Logits are produced transposed (lhsT = K^T block), exp on ScalarE,
  causal tail masks on GpSimd.
- AV runs transposed: out^T[d,q] = sum_j V_j^T @ P_j^T accumulated in one
  PSUM bank -- no PE transposes.  The softmax denominator comes from M=1
  ones-matmuls over the masked exp tiles; 1/den is broadcast via matmul
  and fused into the PSUM->SBUF copy as a single tensor_mul.
- The head loop software-pipelines Q-proj of head h+2 between the logits
  and AV phases of head h to keep TensorE dense (HAM stays warm).
- All DRAM->SBUF loads are contiguous per partition (host pre-permutes).
"""

import functools

import numpy as np
import ml_dtypes

import concourse.bass as bass
import concourse.mybir as mybir
import concourse.tile as tile
from concourse.bass_utils import run_bass_kernel_spmd
from concourse.vector_clock import ScopedClock

BF16 = mybir.dt.bfloat16
F32 = mybir.dt.float32
AF = mybir.ActivationFunctionType

B, T, C = 4, 1024, 2048
NH, NKV, D = 16, 4, 128
EPS = 1e-6
SCALE = 1.0 / float(np.sqrt(D))
P = 128
KT = C // P  # 16 contraction tiles
NQB = 4  # q-blocks per core
QIDX = [[0, 3, 4, 7], [1, 2, 5, 6]]  # abs q-block ids per parity
NCORES = 8
NJ = 2 * NQB  # kv blocks handled per core (budget of last slot)


_MAXW = 1  # walrus codegen rejects instructions with more sync waits


class _SplitDrainTileContext(tile.TileContext):
    """Walrus caps sync waits per instruction; Tile's exit drain waits on
    the whole global clock.  Split excess waits onto same-engine nops."""

    def _add_instruction(self, inst):
        si = inst.sync_info
        if si is not None and si.on_wait and len(si.on_wait) > _MAXW:
            waits = list(si.on_wait)
            excess, keep = waits[:-_MAXW], waits[-_MAXW:]
            for i in range(0, len(excess), _MAXW):
                nop = mybir.InstNoOp(
                    name=self.nc.get_next_instruction_name(), ins=[], outs=[]
                )
                nop.engine = inst.engine
                nop.sync_info = mybir.SyncInfo(
                    on_wait=excess[i:i + _MAXW], on_update=[]
                )
                super()._add_instruction(nop)
            inst.sync_info = mybir.SyncInfo(
                on_wait=keep,
                on_update=list(si.on_update) if si.on_update else [],
            )
        super()._add_instruction(inst)

    def _drain_and_barrier(self, tick_clock, wait_clock):
        nc = self.nc
        nops = [nc.sync.nop() for _ in range(40)]
        drain_inst = nc.sync.drain()
        wait_clock.add_sem_waits(
            drain_inst.ins, ScopedClock({None: tick_clock.global_clock})
        )
        si = drain_inst.ins.sync_info
        waits = list(si.on_wait) if si is not None and si.on_wait else []
        if waits:
            assert len(waits) <= len(nops)
            for nop, w in zip(nops, waits):
                nsi = nop.ins.sync_info
                nop.ins.sync_info = mybir.SyncInfo(
                    on_wait=[w],
                    on_update=list(nsi.on_update) if nsi and nsi.on_update else [],
                )
            drain_inst.ins.sync_info = mybir.SyncInfo(
                on_wait=[],
                on_update=list(si.on_update) if si and si.on_update else [],
            )
        nc.all_engine_barrier()
        assert self.sems is not None
        popped = nc._tile_sem_poison_stack.pop()
        assert popped is self._sem_poison
        nc.clear_and_free_semaphores(list(self.sems.allocated().values()))
        nc.all_engine_barrier()


@functools.lru_cache(maxsize=1)
def _build():
    nc = bass.Bass()
    # host pre-permutes everything so each DMA is linear per partition
    xq = nc.declare_dram_parameter("xq", [P, KT, 512], BF16, isOutput=False)
    xkv = nc.declare_dram_parameter("xkv", [P, KT, T], BF16, isOutput=False)
    wq = nc.declare_dram_parameter("wq", [NH, P, KT * P], BF16, isOutput=False)
    wk = nc.declare_dram_parameter("wk", [NKV, P, KT * P], BF16, isOutput=False)
    wv = nc.declare_dram_parameter("wv", [P, KT, NKV * D], BF16, isOutput=False)
    wo = nc.declare_dram_parameter("wo", [4, P, NH, 512], BF16, isOutput=False)
    cfq = nc.declare_dram_parameter("cfq", [P, 512], BF16, isOutput=False)
    sfq = nc.declare_dram_parameter("sfq", [P, 512], BF16, isOutput=False)
    cfk = nc.declare_dram_parameter("cfk", [P, T], BF16, isOutput=False)
    sfk = nc.declare_dram_parameter("sfk", [P, T], BF16, isOutput=False)
    # mask[:, j, :]: 0/1 for (kv tok 128j+p) <= (abs q tok of local col c
    # in q-slot j//2)
    mask2 = nc.declare_dram_parameter("mask2", [P, NJ, P], BF16, isOutput=False)
    out = nc.declare_dram_parameter("out", [512, C], F32, isOutput=True)

    with _SplitDrainTileContext(nc) as tc:
        with (
            tc.tile_pool(name="const", bufs=1) as const,
            tc.tile_pool(name="big", bufs=1) as big,
            tc.tile_pool(name="wqp", bufs=2) as wqp,
            tc.tile_pool(name="wop", bufs=2) as wop,
            tc.tile_pool(name="qhp", bufs=3) as qhp,
            tc.tile_pool(name="tmp", bufs=2) as tmp,
            tc.tile_pool(name="ptp", bufs=12) as ptp,
            tc.tile_pool(name="ps", bufs=1, space="PSUM") as psp,
        ):
            def bank(nm):
                return psp.tile([P, 512], F32, tag="bank", name=nm, bufs=6)

            def rowacc(nm):
                return psp.tile([1, 512], F32, tag="row", name=nm, bufs=2)

            ones_col = const.tile([P, 1], BF16)
            nc.vector.memset(ones_col, 1.0)
            ones_row = const.tile([1, P], BF16)
            nc.vector.memset(ones_row, 1.0)
            eps_sb = const.tile([1, 1], F32)
            nc.vector.memset(eps_sb, EPS)

            # ---- persistent SBUF tensors; DMAs issued in priority order ----
            # (K proj needs wk + xkv first; everything else trails)
            # split large loads across DMA queues (a single queue moves only
            # ~45 GB/s; the aggregate needs many queues)
            wk_sb = big.tile([P, NKV, KT, P], BF16)
            for h in range(NKV):
                nc.sync.dma_start(
                    out=wk_sb[:, h],
                    in_=wk[h].rearrange("p (kt m) -> p kt m", kt=KT),
                )
            xkv_sb = big.tile([P, KT, T], BF16)
            for cc in range(8):
                nc.sync.dma_start(
                    out=xkv_sb[:, 2 * cc:2 * cc + 2, :],
                    in_=xkv[:, 2 * cc:2 * cc + 2, :],
                )
            ck_sb = const.tile([P, T], BF16)
            nc.sync.dma_start(out=ck_sb, in_=cfk[:, :])
            sk_sb = const.tile([P, T], BF16)
            nc.sync.dma_start(out=sk_sb, in_=sfk[:, :])
            cq_sb = const.tile([P, 512], BF16)
            nc.sync.dma_start(out=cq_sb, in_=cfq[:, :])
            sq_sb = const.tile([P, 512], BF16)
            nc.sync.dma_start(out=sq_sb, in_=sfq[:, :])
            mask_sb = const.tile([P, NJ, P], BF16)
            nc.sync.dma_start(out=mask_sb, in_=mask2[:, :, :])
            wv_sb = big.tile([P, KT, NKV * D], BF16)
            for cc in range(4):
                nc.sync.dma_start(
                    out=wv_sb[:, 4 * cc:4 * cc + 4, :],
                    in_=wv[:, 4 * cc:4 * cc + 4, :],
                )
            xq_sb = big.tile([P, KT, 512], BF16)
            for cc in range(4):
                nc.sync.dma_start(
                    out=xq_sb[:, 4 * cc:4 * cc + 4, :],
                    in_=xq[:, 4 * cc:4 * cc + 4, :],
                )

            ktn = big.tile([P, NKV, T], BF16)  # K^T rms-normed+rope [d, kvh, t]
            vtn = big.tile([P, NJ, NKV * D], BF16)  # V [kv tok, j, (kvh d)]
            otn = big.tile([P, NH, 512], BF16)  # attention out^T [d, head, q]

            # ---- shared projection + rope + rmsnorm unit, split A/B so the
            # ss/rb matmuls never sit in front of ready MMs in the in-order
            # PE queue while the scalar chain (square -> ln -> exp) runs ----
            def proj_a(w3, rhs3):
                """phase A: accumulate W.T @ x, stage to SBUF, square it.
                The SBUF copy frees the PSUM bank early, moves the square
                off the scalar engine (gpsimd can't read PSUM) and lets the
                rope tensor_tensors run in 16-bit 2x mode."""
                ps = bank("proj")
                for k in range(KT):
                    nc.tensor.matmul(
                        ps, lhsT=w3[:, k, :], rhs=rhs3[:, k, :],
                        start=(k == 0), stop=(k == KT - 1),
                    )
                c0 = tmp.tile([P, 512], BF16, tag="c0", name="c0", bufs=4)
                nc.vector.tensor_copy(out=c0, in_=ps)
                sq = tmp.tile([P, 512], BF16, tag="sq", name="sq")
                nc.gpsimd.tensor_mul(sq, c0, c0)
                return (ps, c0), sq

            def proj_b1(sq):
                """phase B1: rmsnorm row-sum + rsqrt chain (scalar)."""
                ss = rowacc("ss")
                nc.tensor.matmul(ss, lhsT=ones_col, rhs=sq, start=True, stop=True)
                # rsqrt(m) = exp(-0.5 ln(m)) on the ACT LUTs (Rsqrt is banned,
                # custom DVE ops break this walrus build)
                lnm = tmp.tile([1, 512], F32, tag="lnm", name="lnm")
                nc.scalar.activation(lnm, ss, AF.Ln, bias=eps_sb, scale=1.0 / D)
                rs16 = tmp.tile([1, 512], BF16, tag="rs16", name="rs16")
                nc.scalar.activation(rs16, lnm, AF.Exp, scale=-0.5)
                return rs16

            def proj_b2(psc, rs16, cosT, sinT, out_cols):
                """phase B2: rb broadcast + rope -> out_cols[d, 512].
                The half-swapped muls must read PSUM (DVE cross-partition
                reads are PSUM-source only); m1 reads the bf16 copy at 2x."""
                ps, c0 = psc
                rb = bank("rb")
                nc.tensor.matmul(rb, lhsT=ones_row, rhs=rs16, start=True, stop=True)
                h = D // 2
                m1 = tmp.tile([P, 512], BF16, tag="m1", name="m1")
                m2 = tmp.tile([P, 512], BF16, tag="m2", name="m2")
                t2 = tmp.tile([P, 512], BF16, tag="t2", name="t2")
                nc.vector.tensor_mul(m1, c0, cosT)
                nc.vector.tensor_mul(m2[0:h, :], ps[h:P, :], sinT[0:h, :])
                nc.vector.tensor_mul(m2[h:P, :], ps[0:h, :], sinT[h:P, :])
                nc.vector.tensor_add(t2, m1, m2)
                nc.vector.tensor_mul(out_cols, t2, rb)

            # ---- K projection (4 kv heads x 2 token halves), 2-deep pipe:
            # after A(i) emit B1(i-1) then B2(i-2) so the rb matmul never
            # waits on the scalar ln/exp chain in the in-order PE queue ----
            kjobs = []  # [ps, sq, rs16, t, h]
            def k_b1(i):
                kjobs[i][2] = proj_b1(kjobs[i][1])

            def k_b2(i):
                ps, _, rs16, t, h = kjobs[i]
                sl = slice(t * 512, (t + 1) * 512)
                proj_b2(ps, rs16, ck_sb[:, sl], sk_sb[:, sl], ktn[:, h, sl])

            pairs = [(h, t) for h in range(NKV) for t in range(2)]
            for i, (h, t) in enumerate(pairs):
                ps, sq = proj_a(wk_sb[:, h], xkv_sb[:, :, t * 512:(t + 1) * 512])
                kjobs.append([ps, sq, None, t, h])
                if i >= 1:
                    k_b1(i - 1)
                if i >= 2:
                    k_b2(i - 2)
            k_b1(7)
            k_b2(6)
            k_b2(7)

            # ---- V projection: [tok, vdim] orientation ----
            for j in range(NJ):
                pv = bank("pv")
                for k in range(KT):
                    nc.tensor.matmul(
                        pv,
                        lhsT=xkv_sb[:, k, j * P:(j + 1) * P],
                        rhs=wv_sb[:, k, :],
                        start=(k == 0), stop=(k == KT - 1),
                    )
                nc.scalar.copy(vtn[:, j, :], pv)

            # ---- Q projection phases (pipelined ahead of attention) ----
            qjobs, qhs = {}, {}
            won_pre = []

            def q_a(h):
                w3 = wqp.tile([P, KT, P], BF16, tag="wqh", name="w3")
                nc.sync.dma_start(
                    out=w3, in_=wq[h].rearrange("p (kt m) -> p kt m", kt=KT)
                )
                qjobs[h] = list(proj_a(w3, xq_sb))

            def q_b1(h):
                qjobs[h][1] = proj_b1(qjobs[h][1])

            def q_b2(h):
                ps, rs16 = qjobs.pop(h)
                qh = qhp.tile([P, 512], BF16, tag="qh", name="qh")
                proj_b2(ps, rs16, cq_sb, sq_sb, qh)
                qhs[h] = qh

            q_a(0)
            q_b1(0)
            q_a(1)
            q_b1(1)
            q_b2(0)

            # ---- attention per head; emission order per iteration h:
            #   B2(h+1) | pl/exp/mask x8 (h) | A(h+2) | den (h) | AV (h) |
            #   1/den + recb (h) | B1(h+2)
            # so every PE op's producer ran at least half an iteration ago ----
            for h in range(NH):
                kvh = h // (NH // NKV)
                if h + 1 < NH:
                    q_b2(h + 1)
                qh = qhs.pop(h)
                ptw = []
                for j in range(NJ):
                    smin = j // 2
                    qn = 512 - smin * P
                    pl = bank("pl")
                    nc.tensor.matmul(
                        pl[:, :qn],
                        lhsT=ktn[:, kvh, j * P:(j + 1) * P],
                        rhs=qh[:, smin * P:512],
                        start=True, stop=True,
                    )
                    w = ptp.tile([P, 512], BF16, tag="ptw", name="w")
                    nc.scalar.activation(w[:, :qn], pl[:, :qn], AF.Exp, scale=SCALE)
                    nc.gpsimd.tensor_mul(w[:, 0:P], w[:, 0:P], mask_sb[:, j, :])
                    ptw.append(w)
                if h + 2 < NH:
                    q_a(h + 2)
                # softmax denominator over kv (partition dim) via M=1 matmuls
                den = rowacc("den")
                for j in range(NJ):
                    qo = (j // 2) * P
                    nc.tensor.matmul(
                        den[:, qo:512], lhsT=ones_col, rhs=ptw[j][:, :512 - qo],
                        start=(j == 0), stop=(j == NJ - 1),
                    )
                # AV transposed: po[d, q] += V_j^T @ P_j^T
                po = bank("po")
                for j in range(NJ):
                    qo = (j // 2) * P
                    nc.tensor.matmul(
                        po[:, qo:512],
                        lhsT=vtn[:, j, kvh * D:(kvh + 1) * D],
                        rhs=ptw[j][:, :512 - qo],
                        start=(j == 0), stop=(j == NJ - 1),
                    )
                # 1/den = exp(-ln(den)) on the ACT LUTs, broadcast via matmul
                lnd = tmp.tile([1, 512], F32, tag="lnd", name="lnd")
                nc.scalar.activation(lnd, den, AF.Ln)
                rd16 = tmp.tile([1, 512], BF16, tag="rd16", name="rd16")
                nc.scalar.activation(rd16, lnd, AF.Exp, scale=-1.0)
                recb = bank("recb")
                nc.tensor.matmul(recb, lhsT=ones_row, rhs=rd16, start=True, stop=True)
                recs = tmp.tile([P, 512], BF16, tag="recs", name="recs")
                nc.vector.tensor_copy(out=recs, in_=recb)
                nc.vector.tensor_mul(otn[:, h, :], po, recs)
                if h + 2 < NH:
                    q_b1(h + 2)
                if h in (10, 12):  # prefetch first wo chunks for out-proj
                    won = wop.tile([P, NH, 512], BF16, tag="won", name="won")
                    nc.sync.dma_start(out=won, in_=wo[(h - 10) // 2])
                    won_pre.append(won)

            # ---- output projection, wo streamed per 512-col chunk ----
            for ncol in range(4):
                if ncol < len(won_pre):
                    won = won_pre[ncol]
                else:
                    won = wop.tile([P, NH, 512], BF16, tag="won", name="won")
                    nc.sync.dma_start(out=won, in_=wo[ncol])
                for s in range(NQB):
                    qs = slice(s * P, (s + 1) * P)
                    pso = bank("pso")
                    for h in range(NH):
                        nc.tensor.matmul(
                            pso,
                            lhsT=otn[:, h, qs],
                            rhs=won[:, h, :],
                            start=(h == 0), stop=(h == NH - 1),
                        )
                    ob = tmp.tile([P, 512], F32, tag="ob", name="ob")
                    nc.scalar.copy(ob, pso)
                    nc.sync.dma_start(
                        out=out[qs, ncol * 512:(ncol + 1) * 512], in_=ob
                    )
    return nc


def _prep_inputs(x, cos, sin, Wq, Wk, Wv, Wo):
    bf = ml_dtypes.bfloat16
    # weight permutes (shared across cores): contiguous per partition
    wq_r = np.ascontiguousarray(
        np.asarray(Wq).reshape(KT, P, NH, P).transpose(2, 1, 0, 3)
    ).reshape(NH, P, KT * P).astype(bf)
    wk_r = np.ascontiguousarray(
        np.asarray(Wk).reshape(KT, P, NKV, P).transpose(2, 1, 0, 3)
    ).reshape(NKV, P, KT * P).astype(bf)
    wv_r = np.ascontiguousarray(
        np.asarray(Wv).reshape(KT, P, NKV * D).transpose(1, 0, 2)
    ).astype(bf)
    wo_r = np.ascontiguousarray(
        np.asarray(Wo).reshape(NH, P, 4, 512).transpose(2, 1, 0, 3)
    ).astype(bf)
    cT = np.asarray(cos).T  # [64, T]
    sT = np.asarray(sin).T
    cfk = np.concatenate([cT, cT], axis=0).astype(bf)  # [128, T]
    sfk = np.concatenate([-sT, sT], axis=0).astype(bf)
    kvpos = np.arange(T)
    in_maps = []
    for c in range(NCORES):
        b, g = c // 2, c % 2
        qidx = np.concatenate([np.arange(p * P, (p + 1) * P) for p in QIDX[g]])
        xkv_r = np.ascontiguousarray(
            np.asarray(x[b]).T.reshape(KT, P, T).transpose(1, 0, 2)
        ).astype(bf)  # [128, KT, T]
        xq_r = np.ascontiguousarray(xkv_r[:, :, qidx])
        # mask[:, j, :]: valid = kv tok (128j+p) <= abs q tok of slot j//2
        m2 = np.zeros((P, NJ, P), np.float32)
        for j in range(NJ):
            qcols = qidx[(j // 2) * P:(j // 2 + 1) * P]
            m2[:, j, :] = kvpos[j * P:(j + 1) * P, None] <= qcols[None, :]
        in_maps.append({
            "xq": xq_r, "xkv": xkv_r,
            "wq": wq_r, "wk": wk_r, "wv": wv_r, "wo": wo_r,
            "cfq": np.ascontiguousarray(cfk[:, qidx]),
            "sfq": np.ascontiguousarray(sfk[:, qidx]),
            "cfk": cfk, "sfk": sfk,
            "mask2": m2.astype(bf),
        })
    return in_maps


def _scatter(results):
    out = np.empty((B, T, C), np.float32)
    for c in range(NCORES):
        b, g = c // 2, c % 2
        o = results[c]["out"]
        for s, p in enumerate(QIDX[g]):
            out[b, p * P:(p + 1) * P] = o[s * P:(s + 1) * P]
    return out


def kernel(x, cos, sin, Wq, Wk, Wv, Wo):
    nc = _build()
    in_maps = _prep_inputs(x, cos, sin, Wq, Wk, Wv, Wo)
    res = run_bass_kernel_spmd(nc, in_maps, core_ids=list(range(NCORES)))
    return _scatter(res.results)


# revision 27
# speedup vs baseline: 1.1594x; 1.1594x over previous
"""Causal self-attention (GQA + RoPE + QK-RMSNorm) on 8 trn2 NeuronCores.

Sharding: core c -> (batch b = c//2, parity g = c%2).  Each core computes
the outputs for 512 of its batch's 1024 query tokens (4 q-blocks of 128
tokens; parity 0 takes absolute blocks {0,3,4,7}, parity 1 {1,2,5,6}) over
all 16 heads, plus the full K/V for the batch.  No collectives; the host
gathers/scatters rows only.

Engine plan (v2):
- Projections emit Q^T/K^T with head_dim on partitions.  RMSNorm scale is
  one Dsqrt activation (rsqrt via 0.5/sqrt(x/4)) + a broadcast matmul;
  RoPE uses the swap formulation (x*[c;c] + swap(x)*[-s;s]) = 5 DVE ops.
- Logits are produced transposed (lhsT = K^T block), exp on ScalarE,
  causal tail masks on GpSimd.
- AV runs transposed: out^T[d,q] = sum_j V_j^T @ P_j^T accumulated in one
  PSUM bank -- no PE transposes.  The softmax denominator comes from M=1
  ones-matmuls over the masked exp tiles; 1/den is broadcast via matmul
  and fused into the PSUM->SBUF copy as a single tensor_mul.
- The head loop software-pipelines Q-proj of head h+2 between the logits
  and AV phases of head h to keep TensorE dense (HAM stays warm).
- All DRAM->SBUF loads are contiguous per partition (host pre-permutes).
"""

import functools

import numpy as np
import ml_dtypes

import concourse.bass as bass
import concourse.mybir as mybir
import concourse.tile as tile
from concourse.bass_utils import run_bass_kernel_spmd
from concourse.vector_clock import ScopedClock

BF16 = mybir.dt.bfloat16
F32 = mybir.dt.float32
AF = mybir.ActivationFunctionType

B, T, C = 4, 1024, 2048
NH, NKV, D = 16, 4, 128
EPS = 1e-6
SCALE = 1.0 / float(np.sqrt(D))
P = 128
KT = C // P  # 16 contraction tiles
NQB = 4  # q-blocks per core
QIDX = [[0, 3, 4, 7], [1, 2, 5, 6]]  # abs q-block ids per parity
NCORES = 8
NJ = 2 * NQB  # kv blocks handled per core (budget of last slot)


_MAXW = 1  # walrus codegen rejects instructions with more sync waits


class _SplitDrainTileContext(tile.TileContext):
    """Walrus caps sync waits per instruction; Tile's exit drain waits on
    the whole global clock.  Split excess waits onto same-engine nops."""

    def _add_instruction(self, inst):
        si = inst.sync_info
        if si is not None and si.on_wait and len(si.on_wait) > _MAXW:
            waits = list(si.on_wait)
            excess, keep = waits[:-_MAXW], waits[-_MAXW:]
            for i in range(0, len(excess), _MAXW):
                nop = mybir.InstNoOp(
                    name=self.nc.get_next_instruction_name(), ins=[], outs=[]
                )
                nop.engine = inst.engine
                nop.sync_info = mybir.SyncInfo(
                    on_wait=excess[i:i + _MAXW], on_update=[]
                )
                super()._add_instruction(nop)
            inst.sync_info = mybir.SyncInfo(
                on_wait=keep,
                on_update=list(si.on_update) if si.on_update else [],
            )
        super()._add_instruction(inst)

    def _drain_and_barrier(self, tick_clock, wait_clock):
        nc = self.nc
        nops = [nc.sync.nop() for _ in range(40)]
        drain_inst = nc.sync.drain()
        wait_clock.add_sem_waits(
            drain_inst.ins, ScopedClock({None: tick_clock.global_clock})
        )
        si = drain_inst.ins.sync_info
        waits = list(si.on_wait) if si is not None and si.on_wait else []
        if waits:
            assert len(waits) <= len(nops)
            for nop, w in zip(nops, waits):
                nsi = nop.ins.sync_info
                nop.ins.sync_info = mybir.SyncInfo(
                    on_wait=[w],
                    on_update=list(nsi.on_update) if nsi and nsi.on_update else [],
                )
            drain_inst.ins.sync_info = mybir.SyncInfo(
                on_wait=[],
                on_update=list(si.on_update) if si and si.on_update else [],
            )
        nc.all_engine_barrier()
        assert self.sems is not None
        popped = nc._tile_sem_poison_stack.pop()
        assert popped is self._sem_poison
        nc.clear_and_free_semaphores(list(self.sems.allocated().values()))
        nc.all_engine_barrier()


@functools.lru_cache(maxsize=1)
def _build():
    nc = bass.Bass()
    # host pre-permutes everything so each DMA is linear per partition
    xq = nc.declare_dram_parameter("xq", [P, KT, 512], BF16, isOutput=False)
    xkv = nc.declare_dram_parameter("xkv", [P, KT, T], BF16, isOutput=False)
    wq = nc.declare_dram_parameter("wq", [NH, P, KT * P], BF16, isOutput=False)
    wk = nc.declare_dram_parameter("wk", [NKV, P, KT * P], BF16, isOutput=False)
    wv = nc.declare_dram_parameter("wv", [P, KT, NKV * D], BF16, isOutput=False)
    wo = nc.declare_dram_parameter("wo", [4, P, NH, 512], BF16, isOutput=False)
    cfq = nc.declare_dram_parameter("cfq", [P, 512], BF16, isOutput=False)
    sfq = nc.declare_dram_parameter("sfq", [P, 512], BF16, isOutput=False)
    cfk = nc.declare_dram_parameter("cfk", [P, T], BF16, isOutput=False)
    sfk = nc.declare_dram_parameter("sfk", [P, T], BF16, isOutput=False)
    # mask[:, j, :]: 0/1 for (kv tok 128j+p) <= (abs q tok of local col c
    # in q-slot j//2)
    mask2 = nc.declare_dram_parameter("mask2", [P, NJ, P], BF16, isOutput=False)
    out = nc.declare_dram_parameter("out", [512, C], F32, isOutput=True)

    with _SplitDrainTileContext(nc) as tc:
        with (
            tc.tile_pool(name="const", bufs=1) as const,
            tc.tile_pool(name="big", bufs=1) as big,
            tc.tile_pool(name="wqp", bufs=2) as wqp,
            tc.tile_pool(name="wop", bufs=2) as wop,
            tc.tile_pool(name="qhp", bufs=3) as qhp,
            tc.tile_pool(name="tmp", bufs=2) as tmp,
            tc.tile_pool(name="ptp", bufs=12) as ptp,
            tc.tile_pool(name="ps", bufs=1, space="PSUM") as psp,
        ):
            def bank(nm):
                return psp.tile([P, 512], F32, tag="bank", name=nm, bufs=6)

            def rowacc(nm):
                return psp.tile([1, 512], F32, tag="row", name=nm, bufs=2)

            ones_col = const.tile([P, 1], BF16)
            nc.vector.memset(ones_col, 1.0)
            ones_row = const.tile([1, P], BF16)
            nc.vector.memset(ones_row, 1.0)
            eps_sb = const.tile([1, 1], F32)
            nc.vector.memset(eps_sb, EPS)

            # ---- persistent SBUF tensors; DMAs issued in priority order ----
            # (K proj needs wk + xkv first; everything else trails)
            # split large loads across DMA queues (a single queue moves only
            # ~45 GB/s; the aggregate needs many queues)
            wk_sb = big.tile([P, NKV, KT, P], BF16)
            for h in range(NKV):
                for q in range(2):
                    nc.sync.dma_start(
                        out=wk_sb[:, h, 8 * q:8 * q + 8, :],
                        in_=wk[h, :, 1024 * q:1024 * q + 1024].rearrange(
                            "p (kt m) -> p kt m", kt=8
                        ),
                    )
            xkv_sb = big.tile([P, KT, T], BF16)
            for cc in range(16):
                nc.sync.dma_start(
                    out=xkv_sb[:, cc:cc + 1, :], in_=xkv[:, cc:cc + 1, :]
                )
            ck_sb = const.tile([P, T], BF16)
            nc.sync.dma_start(out=ck_sb, in_=cfk[:, :])
            sk_sb = const.tile([P, T], BF16)
            nc.sync.dma_start(out=sk_sb, in_=sfk[:, :])
            cq_sb = const.tile([P, 512], BF16)
            nc.sync.dma_start(out=cq_sb, in_=cfq[:, :])
            sq_sb = const.tile([P, 512], BF16)
            nc.sync.dma_start(out=sq_sb, in_=sfq[:, :])
            mask_sb = const.tile([P, NJ, P], BF16)
            nc.sync.dma_start(out=mask_sb, in_=mask2[:, :, :])
            wv_sb = big.tile([P, KT, NKV * D], BF16)
            for cc in range(4):
                nc.sync.dma_start(
                    out=wv_sb[:, 4 * cc:4 * cc + 4, :],
                    in_=wv[:, 4 * cc:4 * cc + 4, :],
                )
            xq_sb = big.tile([P, KT, 512], BF16)
            for cc in range(4):
                nc.sync.dma_start(
                    out=xq_sb[:, 4 * cc:4 * cc + 4, :],
                    in_=xq[:, 4 * cc:4 * cc + 4, :],
                )

            ktn = big.tile([P, NKV, T], BF16)  # K^T rms-normed+rope [d, kvh, t]
            vtn = big.tile([P, NJ, NKV * D], BF16)  # V [kv tok, j, (kvh d)]
            otn = big.tile([P, NH, 512], BF16)  # attention out^T [d, head, q]

            # ---- shared projection + rope + rmsnorm unit, split A/B so the
            # ss/rb matmuls never sit in front of ready MMs in the in-order
            # PE queue while the scalar chain (square -> ln -> exp) runs ----
            def proj_a(w3, rhs3):
                """phase A: accumulate W.T @ x, stage to SBUF, square it.
                The SBUF copy frees the PSUM bank early, moves the square
                off the scalar engine (gpsimd can't read PSUM) and lets the
                rope tensor_tensors run in 16-bit 2x mode."""
                ps = bank("proj")
                for k in range(KT):
                    nc.tensor.matmul(
                        ps, lhsT=w3[:, k, :], rhs=rhs3[:, k, :],
                        start=(k == 0), stop=(k == KT - 1),
                    )
                sq = tmp.tile([P, 512], BF16, tag="sq", name="sq")
                nc.scalar.square(sq, ps)
                return ps, sq

            def proj_b1(sq):
                """phase B1: rmsnorm row-sum + rsqrt chain (scalar)."""
                ss = rowacc("ss")
                nc.tensor.matmul(ss, lhsT=ones_col, rhs=sq, start=True, stop=True)
                # rsqrt(m) = exp(-0.5 ln(m)) on the ACT LUTs (Rsqrt is banned,
                # custom DVE ops break this walrus build)
                lnm = tmp.tile([1, 512], F32, tag="lnm", name="lnm")
                nc.scalar.activation(lnm, ss, AF.Ln, bias=eps_sb, scale=1.0 / D)
                rs16 = tmp.tile([1, 512], BF16, tag="rs16", name="rs16")
                nc.scalar.activation(rs16, lnm, AF.Exp, scale=-0.5)
                return rs16

            def proj_b2(ps, rs16, cosT, sinT, out_cols):
                """phase B2: rb broadcast + rope -> out_cols[d, 512].
                The half-swapped muls read PSUM (DVE cross-partition reads
                are PSUM-source only)."""
                rb = bank("rb")
                nc.tensor.matmul(rb, lhsT=ones_row, rhs=rs16, start=True, stop=True)
                h = D // 2
                m1 = tmp.tile([P, 512], BF16, tag="m1", name="m1")
                m2 = tmp.tile([P, 512], BF16, tag="m2", name="m2")
                t2 = tmp.tile([P, 512], BF16, tag="t2", name="t2")
                nc.vector.tensor_mul(m1, ps, cosT)
                nc.vector.tensor_mul(m2[0:h, :], ps[h:P, :], sinT[0:h, :])
                nc.vector.tensor_mul(m2[h:P, :], ps[0:h, :], sinT[h:P, :])
                nc.vector.tensor_add(t2, m1, m2)
                nc.vector.tensor_mul(out_cols, t2, rb)

            # ---- K projection (4 kv heads x 2 token halves), 2-deep pipe:
            # after A(i) emit B1(i-1) then B2(i-2) so the rb matmul never
            # waits on the scalar ln/exp chain in the in-order PE queue ----
            kjobs = []  # [ps, sq, rs16, t, h]
            def k_b1(i):
                kjobs[i][2] = proj_b1(kjobs[i][1])

            def k_b2(i):
                ps, _, rs16, t, h = kjobs[i]
                sl = slice(t * 512, (t + 1) * 512)
                proj_b2(ps, rs16, ck_sb[:, sl], sk_sb[:, sl], ktn[:, h, sl])

            pairs = [(h, t) for h in range(NKV) for t in range(2)]
            for i, (h, t) in enumerate(pairs):
                ps, sq = proj_a(wk_sb[:, h], xkv_sb[:, :, t * 512:(t + 1) * 512])
                kjobs.append([ps, sq, None, t, h])
                if i >= 1:
                    k_b1(i - 1)
                if i >= 2:
                    k_b2(i - 2)
            k_b1(7)
            k_b2(6)
            k_b2(7)

            # ---- V projection: [tok, vdim] orientation ----
            for j in range(NJ):
                pv = bank("pv")
                for k in range(KT):
                    nc.tensor.matmul(
                        pv,
                        lhsT=xkv_sb[:, k, j * P:(j + 1) * P],
                        rhs=wv_sb[:, k, :],
                        start=(k == 0), stop=(k == KT - 1),
                    )
                nc.scalar.copy(vtn[:, j, :], pv)

            # ---- Q projection phases (pipelined ahead of attention) ----
            qjobs, qhs = {}, {}
            won_pre = []

            def q_a(h):
                w3 = wqp.tile([P, KT, P], BF16, tag="wqh", name="w3")
                nc.sync.dma_start(
                    out=w3, in_=wq[h].rearrange("p (kt m) -> p kt m", kt=KT)
                )
                qjobs[h] = list(proj_a(w3, xq_sb))

            def q_b1(h):
                qjobs[h][1] = proj_b1(qjobs[h][1])

            def q_b2(h):
                ps, rs16 = qjobs.pop(h)
                qh = qhp.tile([P, 512], BF16, tag="qh", name="qh")
                proj_b2(ps, rs16, cq_sb, sq_sb, qh)
                qhs[h] = qh

            q_a(0)
            q_b1(0)
            q_a(1)
            q_b1(1)
            q_b2(0)

            # ---- attention per head; emission order per iteration h:
            #   B2(h+1) | pl/exp/mask x8 (h) | A(h+2) | den (h) | AV (h) |
            #   1/den + recb (h) | B1(h+2)
            # so every PE op's producer ran at least half an iteration ago ----
            for h in range(NH):
                kvh = h // (NH // NKV)
                if h + 1 < NH:
                    q_b2(h + 1)
                qh = qhs.pop(h)
                ptw = []  # (tile, col base) per kv block j
                for j in range(4):
                    smin = j // 2
                    qn = 512 - smin * P
                    pl = bank("pl")
                    nc.tensor.matmul(
                        pl[:, :qn],
                        lhsT=ktn[:, kvh, j * P:(j + 1) * P],
                        rhs=qh[:, smin * P:512],
                        start=True, stop=True,
                    )
                    w = ptp.tile([P, 512], BF16, tag="ptw", name="w")
                    nc.scalar.activation(w[:, :qn], pl[:, :qn], AF.Exp, scale=SCALE)
                    nc.gpsimd.tensor_mul(w[:, 0:P], w[:, 0:P], mask_sb[:, j, :])
                    ptw.append((w, 0))
                # j=4,5 and j=6,7 pairs share one PSUM bank so each pair
                # needs a single exp activation
                for j0, qn in ((4, 256), (6, 128)):
                    plp = bank("pl")
                    for u in range(2):
                        nc.tensor.matmul(
                            plp[:, u * qn:(u + 1) * qn],
                            lhsT=ktn[:, kvh, (j0 + u) * P:(j0 + u + 1) * P],
                            rhs=qh[:, 512 - qn:512],
                            start=True, stop=True,
                        )
                    w = ptp.tile([P, 512], BF16, tag="ptw", name="w")
                    nc.scalar.activation(
                        w[:, :2 * qn], plp[:, :2 * qn], AF.Exp, scale=SCALE
                    )
                    for u in range(2):
                        nc.gpsimd.tensor_mul(
                            w[:, u * qn:u * qn + P],
                            w[:, u * qn:u * qn + P],
                            mask_sb[:, j0 + u, :],
                        )
                        ptw.append((w, u * qn))
                if h + 2 < NH:
                    q_a(h + 2)
                # softmax denominator over kv (partition dim) via M=1 matmuls
                den = rowacc("den")
                for j in range(NJ):
                    qo = (j // 2) * P
                    w, b = ptw[j]
                    nc.tensor.matmul(
                        den[:, qo:512], lhsT=ones_col, rhs=w[:, b:b + 512 - qo],
                        start=(j == 0), stop=(j == NJ - 1),
                    )
                # AV transposed: po[d, q] += V_j^T @ P_j^T
                po = bank("po")
                for j in range(NJ):
                    qo = (j // 2) * P
                    w, b = ptw[j]
                    nc.tensor.matmul(
                        po[:, qo:512],
                        lhsT=vtn[:, j, kvh * D:(kvh + 1) * D],
                        rhs=w[:, b:b + 512 - qo],
                        start=(j == 0), stop=(j == NJ - 1),
                    )
                # 1/den = exp(-ln(den)) on the ACT LUTs, broadcast via matmul
                lnd = tmp.tile([1, 512], F32, tag="lnd", name="lnd")
                nc.scalar.activation(lnd, den, AF.Ln)
                rd16 = tmp.tile([1, 512], BF16, tag="rd16", name="rd16")
                nc.scalar.activation(rd16, lnd, AF.Exp, scale=-1.0)
                recb = bank("recb")
                nc.tensor.matmul(recb, lhsT=ones_row, rhs=rd16, start=True, stop=True)
                recs = tmp.tile([P, 512], BF16, tag="recs", name="recs")
                nc.vector.tensor_copy(out=recs, in_=recb)
                nc.vector.tensor_mul(otn[:, h, :], po, recs)
                if h + 2 < NH:
                    q_b1(h + 2)
                if h in (10, 12):  # prefetch first wo chunks for out-proj
                    won = wop.tile([P, NH, 512], BF16, tag="won", name="won")
                    nc.sync.dma_start(out=won, in_=wo[(h - 10) // 2])
                    won_pre.append(won)

            # ---- output projection, wo streamed per 512-col chunk ----
            for ncol in range(4):
                if ncol < len(won_pre):
                    won = won_pre[ncol]
                else:
                    won = wop.tile([P, NH, 512], BF16, tag="won", name="won")
                    nc.sync.dma_start(out=won, in_=wo[ncol])
                for s in range(NQB):
                    qs = slice(s * P, (s + 1) * P)
                    pso = bank("pso")
                    for h in range(NH):
                        nc.tensor.matmul(
                            pso,
                            lhsT=otn[:, h, qs],
                            rhs=won[:, h, :],
                            start=(h == 0), stop=(h == NH - 1),
                        )
                    ob = tmp.tile([P, 512], F32, tag="ob", name="ob")
                    nc.scalar.copy(ob, pso)
                    nc.sync.dma_start(
                        out=out[qs, ncol * 512:(ncol + 1) * 512], in_=ob
                    )
    return nc


def _prep_inputs(x, cos, sin, Wq, Wk, Wv, Wo):
    bf = ml_dtypes.bfloat16
    # weight permutes (shared across cores): contiguous per partition
    wq_r = np.ascontiguousarray(
        np.asarray(Wq).reshape(KT, P, NH, P).transpose(2, 1, 0, 3)
    ).reshape(NH, P, KT * P).astype(bf)
    wk_r = np.ascontiguousarray(
        np.asarray(Wk).reshape(KT, P, NKV, P).transpose(2, 1, 0, 3)
    ).reshape(NKV, P, KT * P).astype(bf)
    wv_r = np.ascontiguousarray(
        np.asarray(Wv).reshape(KT, P, NKV * D).transpose(1, 0, 2)
    ).astype(bf)
    wo_r = np.ascontiguousarray(
        np.asarray(Wo).reshape(NH, P, 4, 512).transpose(2, 1, 0, 3)
    ).astype(bf)
    cT = np.asarray(cos).T  # [64, T]
    sT = np.asarray(sin).T
    cfk = np.concatenate([cT, cT], axis=0).astype(bf)  # [128, T]
    sfk = np.concatenate([-sT, sT], axis=0).astype(bf)
    kvpos = np.arange(T)
    in_maps = []
    for c in range(NCORES):
        b, g = c // 2, c % 2
        qidx = np.concatenate([np.arange(p * P, (p + 1) * P) for p in QIDX[g]])
        xkv_r = np.ascontiguousarray(
            np.asarray(x[b]).T.reshape(KT, P, T).transpose(1, 0, 2)
        ).astype(bf)  # [128, KT, T]
        xq_r = np.ascontiguousarray(xkv_r[:, :, qidx])
        # mask[:, j, :]: valid = kv tok (128j+p) <= abs q tok of slot j//2
        m2 = np.zeros((P, NJ, P), np.float32)
        for j in range(NJ):
            qcols = qidx[(j // 2) * P:(j // 2 + 1) * P]
            m2[:, j, :] = kvpos[j * P:(j + 1) * P, None] <= qcols[None, :]
        in_maps.append({
            "xq": xq_r, "xkv": xkv_r,
            "wq": wq_r, "wk": wk_r, "wv": wv_r, "wo": wo_r,
            "cfq": np.ascontiguousarray(cfk[:, qidx]),
            "sfq": np.ascontiguousarray(sfk[:, qidx]),
            "cfk": cfk, "sfk": sfk,
            "mask2": m2.astype(bf),
        })
    return in_maps


def _scatter(results):
    out = np.empty((B, T, C), np.float32)
    for c in range(NCORES):
        b, g = c // 2, c % 2
        o = results[c]["out"]
        for s, p in enumerate(QIDX[g]):
            out[b, p * P:(p + 1) * P] = o[s * P:(s + 1) * P]
    return out


def kernel(x, cos, sin, Wq, Wk, Wv, Wo):
    nc = _build()
    in_maps = _prep_inputs(x, cos, sin, Wq, Wk, Wv, Wo)
    res = run_bass_kernel_spmd(nc, in_maps, core_ids=list(range(NCORES)))
    return _scatter(res.results)
